# revision 17
# baseline (speedup 1.0000x reference)
"""Trainium2 8-core attention kernel v15 (N=8192, D=512, Q==K shared projection).

Projection-free formulation; score GEMM, PV GEMM, the output projection AND
the etlg prep all run fp8e4 DoubleRow.

Host folds the weight-only products G = 64*(W_qk^T W_qk) and
h = 64*(W_qk^T b_qk).  G ships as TWO fp8 levels (G1 = f8(64G),
G2 = f8(64G - G1)): single-level fp8 G has a systematic diagonal-logit
shift (e4m3 step at the Gram-diagonal magnitude), two levels recover it;
both levels accumulate into the same PSUM group (fp8 scale-invariance).
The Exp activation scale absorbs the 1/64.

Scores:  st^T[j, i] = 64 * e_j . (G e_i + h)   (row-constant softmax terms
are dropped -- softmax-invariant).  etlg8 = (G1+G2) E8^T_loc + h 1^T comes
straight from the fp8 et8 slabs (slabs 7,8 = the core's own rows, DMA'd
first), so the score stream is gated by only ~640KB of DMA: startup is
DMA-bound (~50-100 GB/s/queue early), hence G1's matmuls run as soon as it
lands (~10us) with their 4 PSUM groups HELD OPEN until G2 arrives (~13us);
scores start ~15us vs ~22us for the bf16-G-via-etl path.

Value side:  attn @ V = (P @ E) W_v^T + b_v.  The softmax DIAGONAL
(logit ~ 10.2, 70% of the softmax mass) is split out exactly: its logit is
extracted from PSUM (masked accum), then suppressed (-2^17) before the Exp
writes P directly in fp8e4.  (P@E)^T runs fp8 DoubleRow over column-chunk
pairs; the projection runs fp8 DoubleRow too (ptb8 = (P@E)^T/16 evac'd
pairwise, wv8 = 16*W_v^T -- scales cancel); the diagonal contribution
p_ii * (E_loc W_v^T)_i is added back from a bf16 local V projection (kept
bf16: the diag mass makes V_loc precision critical) before normalization.
etl (bf16 local E^T) and wv are only needed for V_loc, which is spread one
128-row chunk at a time through u=14..21 of the rb0 score stream; etlg for
row-block 1 is spread through u=2..5.  The u-loop is SOFTWARE-PIPELINED:
score(u+1) issues before PV(u) so the Exp(u) ACT latency hides under real
matmuls (the PE queue is in-order).

The diagonal lands at compile-time-fixed loop positions on every core by
ROTATING each core's column-chunk order: core c processes global chunk
(q + 8c + 36) % 64 at loop position q, so diag chunks are always
q = 28 + rb*4 + jj.  The et8/en8 host buffers are built in that per-core
order (PV/l-sums are order-invariant).

Queues: Q1 (SWDGE/gpsimd, starts ~3us before HWDGE) carries the merged
small-tensor block (h|BIG*I as one [128,132] transfer -- [128, small] DMAs
cost ~2.7us in per-packet overhead, and stride-0 broadcast sources are far
worse, so bv is pre-broadcast on the host), G1, G2 and the early et8/en8
slabs in consumption order; Q0 (HWDGE/sync) carries wv, etl, the late slabs,
bv and all output tiles.  23 junk matmuls bridge the HAM clock-gate warmup
across the DMA-bound startup window.
"""

import ml_dtypes
import numpy as np

import concourse.bass as bass
import concourse.mybir as mybir
import concourse.tile as tile
from concourse import bacc
from concourse.bass_utils import run_bass_kernel_spmd

N = 8192          # sequence length
F = 512           # input features
D = 512           # output features (head dim)
CORES = 8
NL = N // CORES   # local rows per core (1024)
SCALE = 1.0 / float(np.sqrt(D))
GSC = 64.0        # host-side scaling on G/h
SC2 = SCALE / GSC # Exp activation scale (absorbs the G scaling)
PSC = 16.0        # ptb8 = pvt/16, wv8 = 16*W_v^T (scales cancel)
BIG = 131072.0    # 2**17, diagonal suppression constant

FC = F // 128     # 4 f-chunks
DC = D // 128     # 4 d-chunks
RB = NL // 512    # 2 row-blocks of 512
CC = N // 128     # 64 column chunks
SW = 512          # slab width over N for streamed embedding tensors
NSLAB = N // SW   # 16

f32 = mybir.dt.float32
bf16 = mybir.dt.bfloat16
f8 = mybir.dt.float8e4
DR = mybir.MatmulPerfMode.DoubleRow
DRSW = mybir.MatmulPerfMode.DoubleRowSwInterleave
ACT = mybir.ActivationFunctionType
ALU = mybir.AluOpType

_NC = None
LAST_RESULT = None


def build_kernel():
    nc = bacc.Bacc(target_bir_lowering=False)

    # all in exact SBUF layout, host-prepared (et8/en8 per-core chunk-rotated)
    et8d = nc.declare_dram_parameter("et8", [128, NSLAB * FC * SW], f8, isOutput=False)
    en8d = nc.declare_dram_parameter("en8", [128, CC * F], f8, isOutput=False)
    etld = nc.declare_dram_parameter("etl", [128, RB * FC * SW], bf16, isOutput=False)
    g8ad = nc.declare_dram_parameter("g8a", [128, 2 * 2 * F], f8, isOutput=False)
    g8bd = nc.declare_dram_parameter("g8b", [128, 2 * 2 * F], f8, isOutput=False)
    wvd = nc.declare_dram_parameter("wv", [128, FC * D], bf16, isOutput=False)
    wv8d = nc.declare_dram_parameter("wv8", [128, 2 * 2 * D], f8, isOutput=False)
    smd = nc.declare_dram_parameter("smalls", [128, FC + 128], f32, isOutput=False)
    bvbd = nc.declare_dram_parameter("bvb", [128, D], f32, isOutput=False)
    out = nc.declare_dram_parameter("out", [NL, D], f32, isOutput=True)

    with tile.TileContext(nc) as tc:
        with (
            tc.tile_pool(name="persist", bufs=1) as persist,
            tc.tile_pool(name="work", bufs=2) as work,
            tc.tile_pool(name="ps", bufs=3, space="PSUM") as ps,
        ):
            # ---- HAM warmup: junk matmuls keep PE busy while DMAs land ----
            junk = persist.tile([128, 512], bf16)
            nc.vector.memset(junk, 0.25)
            junk_ps = ps.tile([128, 512], f32, tag="mm_ps")
            for _ in range(26):
                nc.tensor.matmul(junk_ps, junk[:, :128], junk,
                                 start=True, stop=True, skip_group_check=True)

            g8a = persist.tile([128, 2 * 2 * F], f8)   # f8(64G), (g,k,fp,m)
            g8b = persist.tile([128, 2 * 2 * F], f8)   # f8(64G - G1)
            wv = persist.tile([128, FC * D], bf16)     # W_v^T,  f-chunk fc at cols fc*D
            wv8 = persist.tile([128, 2 * 2 * D], f8)   # 16*W_v^T, (g,k,d) DR pairs
            # E^T local, nb-major: (nb, fc) block at cols nb*FC*512 + fc*512
            etl = persist.tile([128, RB * FC * SW], bf16)
            et8 = persist.tile([128, NSLAB * FC * SW], f8)
            en8 = persist.tile([128, CC * F], f8)
            smalls = persist.tile([128, FC + 128], f32)  # h | BIG*identity
            h_d = smalls[:, 0:FC]
            idm = smalls[:, FC:FC + 128]
            bv_bc = persist.tile([128, D], f32)

            def et8_slab(sl):
                nc.gpsimd.dma_start(
                    out=et8[:, sl * FC * SW:(sl + 1) * FC * SW],
                    in_=et8d[:, sl * FC * SW:(sl + 1) * FC * SW])

            def en8_slab(sl):
                nc.gpsimd.dma_start(
                    out=en8[:, sl * FC * SW:(sl + 1) * FC * SW],
                    in_=en8d[:, sl * FC * SW:(sl + 1) * FC * SW])

            # Q1 (SWDGE/gpsimd — starts ~3us earlier than HWDGE): the full
            # score-gating chain: smalls, G1, et8 s7 (etlg rhs nb0), G2, s8,
            # s0, s1, en8 s0, then slabs in consumption order
            nc.gpsimd.dma_start(out=smalls[:, :], in_=smd[:, :])
            nc.gpsimd.dma_start(out=g8a[:, :], in_=g8ad[:, :])
            et8_slab(7)
            nc.gpsimd.dma_start(out=g8b[:, :], in_=g8bd[:, :])
            et8_slab(8)
            et8_slab(0)
            et8_slab(1)
            en8_slab(0)
            et8_slab(2)
            en8_slab(1)
            et8_slab(3)
            en8_slab(2)
            et8_slab(4)
            en8_slab(3)
            en8_slab(4)
            en8_slab(5)
            nc.gpsimd.dma_start(out=wv8[:, :], in_=wv8d[:, :])

            # Q0 (HWDGE/sync — slow start): V_loc inputs, late slabs, bv
            nc.sync.dma_start(out=wv[:, :], in_=wvd[:, :])
            nc.sync.dma_start(out=etl[:, :2048], in_=etld[:, :2048])
            nc.sync.dma_start(out=etl[:, 2048:], in_=etld[:, 2048:])
            for sl in (5, 6, 9, 10):
                nc.sync.dma_start(
                    out=et8[:, sl * FC * SW:(sl + 1) * FC * SW],
                    in_=et8d[:, sl * FC * SW:(sl + 1) * FC * SW])
            for sl in (6, 7):
                nc.sync.dma_start(
                    out=en8[:, sl * FC * SW:(sl + 1) * FC * SW],
                    in_=en8d[:, sl * FC * SW:(sl + 1) * FC * SW])
            nc.sync.dma_start(out=bv_bc, in_=bvbd[:, :])
            for sl in (11, 12, 13, 14, 15):
                nc.sync.dma_start(
                    out=et8[:, sl * FC * SW:(sl + 1) * FC * SW],
                    in_=et8d[:, sl * FC * SW:(sl + 1) * FC * SW])
            for sl in range(8, NSLAB):
                nc.sync.dma_start(
                    out=en8[:, sl * FC * SW:(sl + 1) * FC * SW],
                    in_=en8d[:, sl * FC * SW:(sl + 1) * FC * SW])

            ones_b = persist.tile([128, 1], bf16)
            nc.vector.memset(ones_b, 1.0)

            # ---- prep: etlg8 = (G1+G2) E8^T_loc + h 1^T (fp8 out) ----
            etlg8 = persist.tile([128, FC * NL], f8)    # f-chunk fc at cols fc*NL
            V_nb = persist.tile([128, 8 * D], bf16)     # local V (no bias), ic at ic*D

            def g_lhsT(gt, g, fp):
                return gt[:, g * 2 * F:(g + 1) * 2 * F].rearrange(
                    "p (k n) -> p k n", k=2)[:, :, fp * 128:(fp + 1) * 128]

            def e_rhs(nb, g):
                base = (7 + nb) * FC * SW
                return et8[:, base + 2 * g * SW:
                           base + (2 * g + 2) * SW].rearrange("p (k n) -> p k n", k=2)

            def etlg_evac(ep, nb, fp):
                nc.vector.tensor_scalar_add(
                    out=etlg8[:, fp * NL + nb * 512: fp * NL + nb * 512 + 512],
                    in0=ep, scalar1=h_d[:, fp:fp + 1])

            # nb0: G1 matmuls start as soon as g8a lands; groups stay open
            # until G2 closes them (separate tag: 4 banks held across the wait)
            etlg_ps = [ps.tile([128, 512], f32, tag="pvt_ps", bufs=4,
                               name=f"etlg_ps{fp}") for fp in range(FC)]
            for fp in range(FC):
                for g in range(2):
                    nc.tensor.matmul(etlg_ps[fp], g_lhsT(g8a, g, fp), e_rhs(0, g),
                                     start=(g == 0), stop=False, perf_mode=DR)
            for fp in range(FC):
                for g in range(2):
                    nc.tensor.matmul(etlg_ps[fp], g_lhsT(g8b, g, fp), e_rhs(0, g),
                                     start=False, stop=(g == 1), perf_mode=DR)
                etlg_evac(etlg_ps[fp], 0, fp)

            def emit_etlg_nb1(fp):
                g_ps = ps.tile([128, 512], f32, tag="mm_ps")
                for i, (gt, g) in enumerate(
                        [(g8a, 0), (g8a, 1), (g8b, 0), (g8b, 1)]):
                    nc.tensor.matmul(g_ps, g_lhsT(gt, g, fp), e_rhs(1, g),
                                     start=(i == 0), stop=(i == 3), perf_mode=DR)
                etlg_evac(g_ps, 1, fp)

            def emit_Vnb_ic(ic):
                v_ps = ps.tile([128, 512], f32, tag="mm_ps")
                for fc in range(FC):
                    nc.tensor.matmul(
                        v_ps,
                        etl[:, (ic // 4) * FC * SW + fc * SW + (ic % 4) * 128:
                            (ic // 4) * FC * SW + fc * SW + (ic % 4) * 128 + 128],
                        wv[:, fc * D:(fc + 1) * D],
                        start=(fc == 0), stop=(fc == FC - 1),
                    )
                if ic % 2 == 0:
                    nc.vector.tensor_copy(out=V_nb[:, ic * D:(ic + 1) * D], in_=v_ps)
                else:
                    nc.scalar.activation(out=V_nb[:, ic * D:(ic + 1) * D],
                                         in_=v_ps, func=ACT.Copy)

            # ---- attention: 2 row-blocks of 512 local rows ----
            for rb in range(RB):
                r0 = rb * 512
                pvt_ps = [
                    ps.tile([128, 512], f32, tag="pvt_ps", bufs=4, name=f"pvt{rb}_{fb}")
                    for fb in range(FC)
                ]
                lacc = [work.tile([128, 512], f32, tag="lacc", bufs=4,
                                  name=f"lacc{rb}_{h}") for h in range(2)]
                ppl = work.tile([128, 4], f32, tag="ppl", bufs=2)     # diag logits
                pp4 = work.tile([128, 4], f32, tag="pp4", bufs=2)     # exp(diag)
                dg = [work.tile([128, 128], bf16, tag="dg", bufs=8,
                                name=f"dg_{rb}_{j}") for j in range(4)]
                def emit_score(u):
                    p8 = work.tile([128, 2 * 512], f8, tag="p8", bufs=4)
                    for h in range(2):
                        cc = 2 * u + h
                        sl, t = divmod(cc, FC)
                        st_ps = ps.tile([128, 512], f32, tag="mm_ps")
                        for g in range(2):
                            lhsT = et8[:, sl * FC * SW + 2 * g * SW:
                                       sl * FC * SW + (2 * g + 2) * SW].rearrange(
                                "p (k n) -> p k n", k=2)[:, :, t * 128:(t + 1) * 128]
                            rhs = etlg8[:, 2 * g * NL:(2 * g + 2) * NL].rearrange(
                                "p (k n) -> p k n", k=2)[:, :, r0:r0 + 512]
                            nc.tensor.matmul(
                                st_ps, lhsT, rhs,
                                start=(g == 0), stop=(g == 1), perf_mode=DR,
                            )
                        jj = cc - (28 + rb * 4)
                        if 0 <= jj < 4:
                            # extract diag logit (masked accum), then suppress
                            sli = st_ps[:, jj * 128:(jj + 1) * 128]
                            trash = work.tile([128, 128], f32, tag="trash", bufs=2)
                            nc.vector.scalar_tensor_tensor(
                                out=trash, in0=sli, scalar=1.0 / BIG, in1=idm,
                                op0=ALU.mult, op1=ALU.mult,
                                accum_out=ppl[:, jj:jj + 1],
                            )
                            nc.vector.scalar_tensor_tensor(
                                out=sli, in0=sli, scalar=1.0, in1=idm,
                                op0=ALU.mult, op1=ALU.subtract,
                            )
                        nc.scalar.activation(
                            out=p8[:, h * 512:(h + 1) * 512], in_=st_ps,
                            func=ACT.Exp, scale=SC2,
                        )
                        eng = nc.gpsimd if h == 0 else nc.vector
                        if u == 0:
                            eng.tensor_copy(out=lacc[h], in_=p8[:, h * 512:(h + 1) * 512])
                        else:
                            eng.tensor_add(lacc[h], lacc[h], p8[:, h * 512:(h + 1) * 512])
                        if cc == 31 + rb * 4:
                            nc.scalar.activation(out=pp4, in_=ppl,
                                                 func=ACT.Exp, scale=SC2)
                    return p8

                def emit_pv(u, p8):
                    p8r = p8[:, :].rearrange("p (k n) -> p k n", k=2)
                    for fb in range(FC):
                        base = (u * FC + fb) * 256
                        lhsT = en8[:, base:base + 256].rearrange(
                            "p (c i) -> p c i", i=2)
                        nc.tensor.matmul(
                            pvt_ps[fb], lhsT, p8r,
                            start=(u == 0), stop=(u == CC // 2 - 1), perf_mode=DRSW,
                        )
                    if u == 17 + rb:
                        s2 = work.tile([128, 4], f32, tag="s2", bufs=2)
                        nc.vector.tensor_scalar_mul(out=s2, in0=pp4, scalar1=1.0 / BIG)
                        for j in range(4):
                            nc.vector.tensor_scalar_mul(
                                out=dg[j], in0=idm, scalar1=s2[:, j:j + 1])

                # software-pipelined: score(u+1) issues before PV(u) so the
                # Exp(u) latency hides under score(u+1)'s matmuls (PE queue
                # is in-order)
                prev_p8 = None
                for u in range(CC // 2 + 1):
                    if rb == 0 and 2 <= u < 6:
                        emit_etlg_nb1(u - 2)
                    if rb == 0 and 14 <= u < 22:
                        emit_Vnb_ic(u - 14)
                    p8 = emit_score(u) if u < CC // 2 else None
                    if prev_p8 is not None:
                        emit_pv(u - 1, prev_p8)
                    prev_p8 = p8

                # rb epilogue.  l-sums first (8 tiny matmuls), then
                # (P@E)^T/16 fp8 pair-evacs + fp8 DR W_v projection for all j
                # (keeps PE busy while linv computes on DVE), then
                # normalize+diag-add-back STTs + DMA.
                laccb = [work.tile([128, 512], bf16, tag="laccb", bufs=4,
                                   name=f"laccb{rb}_{h}") for h in range(2)]
                nc.scalar.activation(out=laccb[0], in_=lacc[0], func=ACT.Copy)
                nc.vector.tensor_copy(out=laccb[1], in_=lacc[1])
                l_ps = ps.tile([128, 8], f32, tag="l_ps", bufs=1)
                for half in range(2):
                    for j in range(4):
                        nc.tensor.matmul(
                            l_ps[:, half * 4 + j:half * 4 + j + 1],
                            laccb[half][:, j * 128:(j + 1) * 128],
                            ones_b,
                            start=True, stop=True, skip_group_check=True,
                        )
                lsum = work.tile([128, 4], f32, tag="lsum")
                nc.vector.tensor_add(lsum, l_ps[:, 0:4], pp4)
                nc.vector.tensor_add(lsum, lsum, l_ps[:, 4:8])
                linv = work.tile([128, 4], f32, tag="linv")
                nc.vector.reciprocal(out=linv, in_=lsum)
                for j in range(4):
                    # ptb8[q]: [p, (k, m)] = pvt[(2q+k)*128+p, j*128+m] / 16
                    ptb8 = [
                        work.tile([128, 256], f8, tag="ptb8", bufs=4,
                                  name=f"ptb8_{rb}_{j}_{q}")
                        for q in range(2)
                    ]
                    for q in range(2):
                        for k in range(2):
                            src = pvt_ps[2 * q + k][:, j * 128:(j + 1) * 128]
                            dsl = ptb8[q][:, k * 128:(k + 1) * 128]
                            if k == 0:
                                nc.vector.tensor_scalar_mul(
                                    out=dsl, in0=src, scalar1=1.0 / PSC)
                            else:
                                nc.scalar.activation(
                                    out=dsl, in_=src, func=ACT.Copy,
                                    scale=1.0 / PSC)
                    o_ps = ps.tile([128, D], f32, tag="mm_ps")
                    for q in range(2):
                        nc.tensor.matmul(
                            o_ps,
                            ptb8[q][:, :].rearrange("p (k n) -> p k n", k=2),
                            wv8[:, q * 2 * D:(q + 1) * 2 * D].rearrange(
                                "p (k n) -> p k n", k=2),
                            start=(q == 0), stop=False, perf_mode=DR,
                        )
                    ic = rb * 4 + j
                    nc.tensor.matmul(
                        o_ps, dg[j], V_nb[:, ic * D:(ic + 1) * D],
                        start=False, stop=True,
                    )
                    o_t = work.tile([128, D], f32, tag="o_t", bufs=4)
                    nc.vector.scalar_tensor_tensor(
                        out=o_t, in0=o_ps, scalar=linv[:, j:j + 1],
                        in1=bv_bc, op0=ALU.mult,
                        op1=ALU.add,
                    )
                    nc.sync.dma_start(
                        out=out[r0 + j * 128: r0 + (j + 1) * 128, :], in_=o_t)

    nc.compile()
    return nc


def _get_nc():
    global _NC
    if _NC is None:
        _NC = build_kernel()
    return _NC


def kernel(embedding, W_qk, b_qk, W_v, b_v):
    global LAST_RESULT
    E = np.ascontiguousarray(np.asarray(embedding, dtype=np.float32))  # [N, F]
    E8 = E.astype(ml_dtypes.float8_e4m3fn)
    chunks = E8.reshape(CC, 128, F)            # (G, p, f) global column chunks

    def prep_w(M):
        M = np.ascontiguousarray(np.asarray(M, dtype=np.float32)).astype(ml_dtypes.bfloat16)
        return np.ascontiguousarray(
            M.reshape(4, 128, M.shape[1]).transpose(1, 0, 2).reshape(128, 4 * M.shape[1]))

    def prep_g(Gm):
        # [p, (g, k, fp, m)] = Gm[fp*128+m, (2g+k)*128+p]
        return np.ascontiguousarray(
            Gm.reshape(FC, 128, 2, 2, 128)      # (fp, m, g, k, p)
            .transpose(4, 2, 3, 0, 1)
            .reshape(128, 2 * 2 * F))

    wqk_f = np.ascontiguousarray(np.asarray(W_qk, dtype=np.float32))
    # weight-only folds: G = 64 * W_qk^T W_qk, h = 64 * W_qk^T b_qk;
    # G ships as two fp8 levels (see module docstring)
    Gm = GSC * (wqk_f.T @ wqk_f)
    G1 = prep_g(Gm).astype(ml_dtypes.float8_e4m3fn)
    G2 = (prep_g(Gm) - G1.astype(np.float32)).astype(ml_dtypes.float8_e4m3fn)
    hv = GSC * (wqk_f.T @ np.asarray(b_qk, dtype=np.float32))
    wvT = np.ascontiguousarray(np.asarray(W_v, dtype=np.float32).T)   # [F, D]
    wv = prep_w(wvT)
    # wv8: [p, (g, k, d)] = 16 * W_v^T[(2g+k)*128+p, d]
    wv8 = np.ascontiguousarray(
        (PSC * wvT).reshape(2, 2, 128, D)       # (g, k, p, d)
        .transpose(2, 0, 1, 3)
        .reshape(128, 2 * 2 * D)).astype(ml_dtypes.float8_e4m3fn)
    bvb = np.ascontiguousarray(
        np.broadcast_to(np.asarray(b_v, dtype=np.float32)[None, :], (128, D)).copy())
    smalls = np.ascontiguousarray(np.concatenate(
        [hv.reshape(FC, 128).T, BIG * np.eye(128, dtype=np.float32)],
        axis=1).astype(np.float32))

    Eb = E.astype(ml_dtypes.bfloat16)
    in_maps = []
    for c in range(CORES):
        order = (np.arange(CC) + 8 * c + 36) % CC
        rot = chunks[order]                     # (q, p, f)
        # et8: [p, (s, fc, q%4, tt)] = E[G(q)*128 + tt, fc*128 + p]
        et8 = np.ascontiguousarray(
            rot.reshape(NSLAB, 4, 128, FC, 128)  # (s, qm, tt, fc, p)
            .transpose(4, 0, 3, 1, 2)
            .reshape(128, NSLAB * FC * SW))
        # en8 (SwInterleave PV stationary): [p, (u, fb, c, i)] =
        #   E[G(2u+i)*128 + p, fb*128 + (127-c)]
        en8 = np.ascontiguousarray(
            rot.reshape(32, 2, 128, FC, 128)[..., ::-1]  # (u, i, p, fb, c)
            .transpose(2, 0, 3, 4, 1)                    # (p, u, fb, c, i)
            .reshape(128, CC * F))
        # etl: [128, (nb, fc, r)] = E[c*NL + nb*512 + r, fc*128+p] in bf16
        etl = np.ascontiguousarray(
            Eb[c * NL:(c + 1) * NL]
            .reshape(RB, SW, FC, 128)
            .transpose(3, 0, 2, 1)
            .reshape(128, RB * FC * SW))
        in_maps.append({
            "et8": et8, "en8": en8, "etl": etl,
            "g8a": G1, "g8b": G2, "wv": wv, "wv8": wv8,
            "smalls": smalls, "bvb": bvb,
        })

    nc = _get_nc()
    res = run_bass_kernel_spmd(nc, in_maps, core_ids=list(range(CORES)))
    LAST_RESULT = res
    return np.concatenate(
        [np.asarray(res.results[i]["out"]) for i in range(CORES)], axis=0
    )


# revision 18
# speedup vs baseline: 1.0136x; 1.0136x over previous
"""Trainium2 8-core attention kernel v15 (N=8192, D=512, Q==K shared projection).

Projection-free formulation; score GEMM, PV GEMM, the output projection AND
the etlg prep all run fp8e4 DoubleRow.

Host folds the weight-only products G = 64*(W_qk^T W_qk) and
h = 64*(W_qk^T b_qk).  G ships as TWO fp8 levels (G1 = f8(64G),
G2 = f8(64G - G1)): single-level fp8 G has a systematic diagonal-logit
shift (e4m3 step at the Gram-diagonal magnitude), two levels recover it;
both levels accumulate into the same PSUM group (fp8 scale-invariance).
The Exp activation scale absorbs the 1/64.

Scores:  st^T[j, i] = 64 * e_j . (G e_i + h)   (row-constant softmax terms
are dropped -- softmax-invariant).  etlg8 = (G1+G2) E8^T_loc + h 1^T comes
straight from the fp8 et8 slabs (slabs 7,8 = the core's own rows, DMA'd
first), so the score stream is gated by only ~640KB of DMA: startup is
DMA-bound (~50-100 GB/s/queue early), hence G1's matmuls run as soon as it
lands (~10us) with their 4 PSUM groups HELD OPEN until G2 arrives (~13us);
scores start ~15us vs ~22us for the bf16-G-via-etl path.

Value side:  attn @ V = (P @ E) W_v^T + b_v.  The softmax DIAGONAL
(logit ~ 10.2, 70% of the softmax mass) is split out exactly: its logit is
extracted from PSUM (masked accum), then suppressed (-2^17) before the Exp
writes P directly in fp8e4.  (P@E)^T runs fp8 DoubleRow over column-chunk
pairs; the projection runs fp8 DoubleRow too (ptb8 = (P@E)^T/16 evac'd
pairwise, wv8 = 16*W_v^T -- scales cancel); the diagonal contribution
p_ii * (E_loc W_v^T)_i is added back from a bf16 local V projection (kept
bf16: the diag mass makes V_loc precision critical) before normalization.
etl (bf16 local E^T) and wv are only needed for V_loc, which is spread one
128-row chunk at a time through u=14..21 of the rb0 score stream; etlg for
row-block 1 is spread through u=2..5.  The u-loop is SOFTWARE-PIPELINED:
score(u+1) issues before PV(u) so the Exp(u) ACT latency hides under real
matmuls (the PE queue is in-order).

The diagonal lands at compile-time-fixed loop positions on every core by
ROTATING each core's column-chunk order: core c processes global chunk
(q + 8c + 36) % 64 at loop position q, so diag chunks are always
q = 28 + rb*4 + jj.  The et8/en8 host buffers are built in that per-core
order (PV/l-sums are order-invariant).

Queues: Q1 (SWDGE/gpsimd, starts ~3us before HWDGE) carries the merged
small-tensor block (h|BIG*I as one [128,132] transfer -- [128, small] DMAs
cost ~2.7us in per-packet overhead, and stride-0 broadcast sources are far
worse, so bv is pre-broadcast on the host), G1, G2 and the early et8/en8
slabs in consumption order; Q0 (HWDGE/sync) carries wv, etl, the late slabs,
bv and all output tiles.  23 junk matmuls bridge the HAM clock-gate warmup
across the DMA-bound startup window.
"""

import ml_dtypes
import numpy as np

import concourse.bass as bass
import concourse.mybir as mybir
import concourse.tile as tile
from concourse import bacc
from concourse.bass_utils import run_bass_kernel_spmd

N = 8192          # sequence length
F = 512           # input features
D = 512           # output features (head dim)
CORES = 8
NL = N // CORES   # local rows per core (1024)
SCALE = 1.0 / float(np.sqrt(D))
GSC = 64.0        # host-side scaling on G/h
SC2 = SCALE / GSC # Exp activation scale (absorbs the G scaling)
PSC = 16.0        # ptb8 = pvt/16, wv8 = 16*W_v^T (scales cancel)
BIG = 131072.0    # 2**17, diagonal suppression constant

FC = F // 128     # 4 f-chunks
DC = D // 128     # 4 d-chunks
RB = NL // 512    # 2 row-blocks of 512
CC = N // 128     # 64 column chunks
SW = 512          # slab width over N for streamed embedding tensors
NSLAB = N // SW   # 16

f32 = mybir.dt.float32
bf16 = mybir.dt.bfloat16
f8 = mybir.dt.float8e4
DR = mybir.MatmulPerfMode.DoubleRow
DRSW = mybir.MatmulPerfMode.DoubleRowSwInterleave
ACT = mybir.ActivationFunctionType
ALU = mybir.AluOpType

_NC = None
LAST_RESULT = None


def build_kernel():
    nc = bacc.Bacc(target_bir_lowering=False)

    # all in exact SBUF layout, host-prepared (et8/en8 per-core chunk-rotated)
    et8d = nc.declare_dram_parameter("et8", [128, NSLAB * FC * SW], f8, isOutput=False)
    en8d = nc.declare_dram_parameter("en8", [128, CC * F], f8, isOutput=False)
    etld = nc.declare_dram_parameter("etl", [128, RB * FC * SW], bf16, isOutput=False)
    g8ad = nc.declare_dram_parameter("g8a", [128, 2 * 2 * F], f8, isOutput=False)
    g8bd = nc.declare_dram_parameter("g8b", [128, 2 * 2 * F], f8, isOutput=False)
    wvd = nc.declare_dram_parameter("wv", [128, FC * D], bf16, isOutput=False)
    wv8d = nc.declare_dram_parameter("wv8", [128, 2 * 2 * D], f8, isOutput=False)
    smd = nc.declare_dram_parameter("smalls", [128, FC + 128], f32, isOutput=False)
    bvbd = nc.declare_dram_parameter("bvb", [128, D], f32, isOutput=False)
    out = nc.declare_dram_parameter("out", [NL, D], f32, isOutput=True)

    with tile.TileContext(nc) as tc:
        with (
            tc.tile_pool(name="persist", bufs=1) as persist,
            tc.tile_pool(name="work", bufs=2) as work,
            tc.tile_pool(name="ps", bufs=3, space="PSUM") as ps,
        ):
            # ---- HAM warmup: junk matmuls keep PE busy while DMAs land ----
            junk = persist.tile([128, 512], bf16)
            nc.vector.memset(junk, 0.25)
            junk_ps = ps.tile([128, 512], f32, tag="mm_ps")
            for _ in range(10):
                nc.tensor.matmul(junk_ps, junk[:, :128], junk,
                                 start=True, stop=True, skip_group_check=True)

            g8a = persist.tile([128, 2 * 2 * F], f8)   # f8(64G), (g,k,fp,m)
            g8b = persist.tile([128, 2 * 2 * F], f8)   # f8(64G - G1)
            wv = persist.tile([128, FC * D], bf16)     # W_v^T,  f-chunk fc at cols fc*D
            wv8 = persist.tile([128, 2 * 2 * D], f8)   # 16*W_v^T, (g,k,d) DR pairs
            # E^T local, nb-major: (nb, fc) block at cols nb*FC*512 + fc*512
            etl = persist.tile([128, RB * FC * SW], bf16)
            et8 = persist.tile([128, NSLAB * FC * SW], f8)
            en8 = persist.tile([128, CC * F], f8)
            smalls = persist.tile([128, FC + 128], f32)  # h | BIG*identity
            h_d = smalls[:, 0:FC]
            idm = smalls[:, FC:FC + 128]
            bv_bc = persist.tile([128, D], f32)

            def et8_slab(sl):
                nc.gpsimd.dma_start(
                    out=et8[:, sl * FC * SW:(sl + 1) * FC * SW],
                    in_=et8d[:, sl * FC * SW:(sl + 1) * FC * SW])

            def en8_slab(sl):
                nc.gpsimd.dma_start(
                    out=en8[:, sl * FC * SW:(sl + 1) * FC * SW],
                    in_=en8d[:, sl * FC * SW:(sl + 1) * FC * SW])

            # Q1 (SWDGE/gpsimd — starts ~3us earlier than HWDGE): the full
            # score-gating chain: smalls, G1, et8 s7 (etlg rhs nb0), G2, s8,
            # s0, s1, en8 s0, then slabs in consumption order
            nc.gpsimd.dma_start(out=g8a[:, :], in_=g8ad[:, :])
            et8_slab(7)
            nc.gpsimd.dma_start(out=g8b[:, :], in_=g8bd[:, :])
            et8_slab(8)
            en8_slab(0)
            et8_slab(2)
            en8_slab(1)
            et8_slab(3)
            en8_slab(2)
            et8_slab(4)
            en8_slab(3)
            en8_slab(4)
            en8_slab(5)
            nc.gpsimd.dma_start(out=wv8[:, :], in_=wv8d[:, :])

            # Q0 (HWDGE/sync — slow start): V_loc inputs, late slabs, bv
            for sl in (0, 1):
                nc.sync.dma_start(
                    out=et8[:, sl * FC * SW:(sl + 1) * FC * SW],
                    in_=et8d[:, sl * FC * SW:(sl + 1) * FC * SW])
            nc.sync.dma_start(out=smalls[:, :], in_=smd[:, :])
            nc.sync.dma_start(out=wv[:, :], in_=wvd[:, :])
            nc.sync.dma_start(out=etl[:, :2048], in_=etld[:, :2048])
            nc.sync.dma_start(out=etl[:, 2048:], in_=etld[:, 2048:])
            for sl in (5, 6, 9, 10):
                nc.sync.dma_start(
                    out=et8[:, sl * FC * SW:(sl + 1) * FC * SW],
                    in_=et8d[:, sl * FC * SW:(sl + 1) * FC * SW])
            for sl in (6, 7):
                nc.sync.dma_start(
                    out=en8[:, sl * FC * SW:(sl + 1) * FC * SW],
                    in_=en8d[:, sl * FC * SW:(sl + 1) * FC * SW])
            nc.sync.dma_start(out=bv_bc, in_=bvbd[:, :])
            for sl in (11, 12, 13, 14, 15):
                nc.sync.dma_start(
                    out=et8[:, sl * FC * SW:(sl + 1) * FC * SW],
                    in_=et8d[:, sl * FC * SW:(sl + 1) * FC * SW])
            for sl in range(8, NSLAB):
                nc.sync.dma_start(
                    out=en8[:, sl * FC * SW:(sl + 1) * FC * SW],
                    in_=en8d[:, sl * FC * SW:(sl + 1) * FC * SW])

            ones_b = persist.tile([128, 1], bf16)
            nc.vector.memset(ones_b, 1.0)

            # ---- prep: etlg8 = (G1+G2) E8^T_loc + h 1^T (fp8 out) ----
            etlg8 = persist.tile([128, FC * NL], f8)    # f-chunk fc at cols fc*NL
            V_nb = persist.tile([128, 8 * D], bf16)     # local V (no bias), ic at ic*D

            def g_lhsT(gt, g, fp):
                return gt[:, g * 2 * F:(g + 1) * 2 * F].rearrange(
                    "p (k n) -> p k n", k=2)[:, :, fp * 128:(fp + 1) * 128]

            def e_rhs(nb, g):
                base = (7 + nb) * FC * SW
                return et8[:, base + 2 * g * SW:
                           base + (2 * g + 2) * SW].rearrange("p (k n) -> p k n", k=2)

            def etlg_evac(ep, nb, fp):
                nc.vector.tensor_scalar_add(
                    out=etlg8[:, fp * NL + nb * 512: fp * NL + nb * 512 + 512],
                    in0=ep, scalar1=h_d[:, fp:fp + 1])

            # nb0: G1 matmuls start as soon as g8a lands; groups stay open
            # until G2 closes them (separate tag: 4 banks held across the wait)
            etlg_ps = [ps.tile([128, 512], f32, tag="pvt_ps", bufs=4,
                               name=f"etlg_ps{fp}") for fp in range(FC)]
            for fp in range(FC):
                for g in range(2):
                    nc.tensor.matmul(etlg_ps[fp], g_lhsT(g8a, g, fp), e_rhs(0, g),
                                     start=(g == 0), stop=False, perf_mode=DR)
            for fp in range(FC):
                for g in range(2):
                    nc.tensor.matmul(etlg_ps[fp], g_lhsT(g8b, g, fp), e_rhs(0, g),
                                     start=False, stop=(g == 1), perf_mode=DR)
                etlg_evac(etlg_ps[fp], 0, fp)

            def emit_etlg_nb1(fp):
                g_ps = ps.tile([128, 512], f32, tag="mm_ps")
                for i, (gt, g) in enumerate(
                        [(g8a, 0), (g8a, 1), (g8b, 0), (g8b, 1)]):
                    nc.tensor.matmul(g_ps, g_lhsT(gt, g, fp), e_rhs(1, g),
                                     start=(i == 0), stop=(i == 3), perf_mode=DR)
                etlg_evac(g_ps, 1, fp)

            def emit_Vnb_ic(ic):
                v_ps = ps.tile([128, 512], f32, tag="mm_ps")
                for fc in range(FC):
                    nc.tensor.matmul(
                        v_ps,
                        etl[:, (ic // 4) * FC * SW + fc * SW + (ic % 4) * 128:
                            (ic // 4) * FC * SW + fc * SW + (ic % 4) * 128 + 128],
                        wv[:, fc * D:(fc + 1) * D],
                        start=(fc == 0), stop=(fc == FC - 1),
                    )
                if ic % 2 == 0:
                    nc.vector.tensor_copy(out=V_nb[:, ic * D:(ic + 1) * D], in_=v_ps)
                else:
                    nc.scalar.activation(out=V_nb[:, ic * D:(ic + 1) * D],
                                         in_=v_ps, func=ACT.Copy)

            # ---- attention: 2 row-blocks of 512 local rows ----
            for rb in range(RB):
                r0 = rb * 512
                pvt_ps = [
                    ps.tile([128, 512], f32, tag="pvt_ps", bufs=4, name=f"pvt{rb}_{fb}")
                    for fb in range(FC)
                ]
                lacc = [work.tile([128, 512], f32, tag="lacc", bufs=4,
                                  name=f"lacc{rb}_{h}") for h in range(2)]
                ppl = work.tile([128, 4], f32, tag="ppl", bufs=2)     # diag logits
                pp4 = work.tile([128, 4], f32, tag="pp4", bufs=2)     # exp(diag)
                dg = [work.tile([128, 128], bf16, tag="dg", bufs=8,
                                name=f"dg_{rb}_{j}") for j in range(4)]
                def emit_score(u):
                    p8 = work.tile([128, 2 * 512], f8, tag="p8", bufs=4)
                    for h in range(2):
                        cc = 2 * u + h
                        sl, t = divmod(cc, FC)
                        st_ps = ps.tile([128, 512], f32, tag="mm_ps")
                        for g in range(2):
                            lhsT = et8[:, sl * FC * SW + 2 * g * SW:
                                       sl * FC * SW + (2 * g + 2) * SW].rearrange(
                                "p (k n) -> p k n", k=2)[:, :, t * 128:(t + 1) * 128]
                            rhs = etlg8[:, 2 * g * NL:(2 * g + 2) * NL].rearrange(
                                "p (k n) -> p k n", k=2)[:, :, r0:r0 + 512]
                            nc.tensor.matmul(
                                st_ps, lhsT, rhs,
                                start=(g == 0), stop=(g == 1), perf_mode=DR,
                            )
                        jj = cc - (28 + rb * 4)
                        if 0 <= jj < 4:
                            # extract diag logit (masked accum), then suppress
                            sli = st_ps[:, jj * 128:(jj + 1) * 128]
                            trash = work.tile([128, 128], f32, tag="trash", bufs=2)
                            nc.vector.scalar_tensor_tensor(
                                out=trash, in0=sli, scalar=1.0 / BIG, in1=idm,
                                op0=ALU.mult, op1=ALU.mult,
                                accum_out=ppl[:, jj:jj + 1],
                            )
                            nc.vector.scalar_tensor_tensor(
                                out=sli, in0=sli, scalar=1.0, in1=idm,
                                op0=ALU.mult, op1=ALU.subtract,
                            )
                        nc.scalar.activation(
                            out=p8[:, h * 512:(h + 1) * 512], in_=st_ps,
                            func=ACT.Exp, scale=SC2,
                        )
                        eng = nc.gpsimd if h == 0 else nc.vector
                        if u == 0:
                            eng.tensor_copy(out=lacc[h], in_=p8[:, h * 512:(h + 1) * 512])
                        else:
                            eng.tensor_add(lacc[h], lacc[h], p8[:, h * 512:(h + 1) * 512])
                        if cc == 31 + rb * 4:
                            nc.scalar.activation(out=pp4, in_=ppl,
                                                 func=ACT.Exp, scale=SC2)
                    return p8

                def emit_pv(u, p8):
                    p8r = p8[:, :].rearrange("p (k n) -> p k n", k=2)
                    for fb in range(FC):
                        base = (u * FC + fb) * 256
                        lhsT = en8[:, base:base + 256].rearrange(
                            "p (c i) -> p c i", i=2)
                        nc.tensor.matmul(
                            pvt_ps[fb], lhsT, p8r,
                            start=(u == 0), stop=(u == CC // 2 - 1), perf_mode=DRSW,
                        )
                    if u == 17 + rb:
                        s2 = work.tile([128, 4], f32, tag="s2", bufs=2)
                        nc.vector.tensor_scalar_mul(out=s2, in0=pp4, scalar1=1.0 / BIG)
                        for j in range(4):
                            nc.vector.tensor_scalar_mul(
                                out=dg[j], in0=idm, scalar1=s2[:, j:j + 1])

                # software-pipelined: score(u+1) issues before PV(u) so the
                # Exp(u) latency hides under score(u+1)'s matmuls (PE queue
                # is in-order)
                prev_p8 = None
                for u in range(CC // 2 + 1):
                    if rb == 0 and 2 <= u < 6:
                        emit_etlg_nb1(u - 2)
                    if rb == 0 and 14 <= u < 22:
                        emit_Vnb_ic(u - 14)
                    p8 = emit_score(u) if u < CC // 2 else None
                    if prev_p8 is not None:
                        emit_pv(u - 1, prev_p8)
                    prev_p8 = p8

                # rb epilogue.  l-sums first (8 tiny matmuls), then
                # (P@E)^T/16 fp8 pair-evacs + fp8 DR W_v projection for all j
                # (keeps PE busy while linv computes on DVE), then
                # normalize+diag-add-back STTs + DMA.
                laccb = [work.tile([128, 512], bf16, tag="laccb", bufs=4,
                                   name=f"laccb{rb}_{h}") for h in range(2)]
                nc.scalar.activation(out=laccb[0], in_=lacc[0], func=ACT.Copy)
                nc.vector.tensor_copy(out=laccb[1], in_=lacc[1])
                l_ps = ps.tile([128, 8], f32, tag="l_ps", bufs=1)
                for half in range(2):
                    for j in range(4):
                        nc.tensor.matmul(
                            l_ps[:, half * 4 + j:half * 4 + j + 1],
                            laccb[half][:, j * 128:(j + 1) * 128],
                            ones_b,
                            start=True, stop=True, skip_group_check=True,
                        )
                lsum = work.tile([128, 4], f32, tag="lsum")
                nc.vector.tensor_add(lsum, l_ps[:, 0:4], pp4)
                nc.vector.tensor_add(lsum, lsum, l_ps[:, 4:8])
                linv = work.tile([128, 4], f32, tag="linv")
                nc.vector.reciprocal(out=linv, in_=lsum)
                for j in range(4):
                    # ptb8[q]: [p, (k, m)] = pvt[(2q+k)*128+p, j*128+m] / 16
                    ptb8 = [
                        work.tile([128, 256], f8, tag="ptb8", bufs=4,
                                  name=f"ptb8_{rb}_{j}_{q}")
                        for q in range(2)
                    ]
                    for q in range(2):
                        for k in range(2):
                            src = pvt_ps[2 * q + k][:, j * 128:(j + 1) * 128]
                            dsl = ptb8[q][:, k * 128:(k + 1) * 128]
                            if k == 0:
                                nc.vector.tensor_scalar_mul(
                                    out=dsl, in0=src, scalar1=1.0 / PSC)
                            else:
                                nc.scalar.activation(
                                    out=dsl, in_=src, func=ACT.Copy,
                                    scale=1.0 / PSC)
                    o_ps = ps.tile([128, D], f32, tag="mm_ps")
                    for q in range(2):
                        nc.tensor.matmul(
                            o_ps,
                            ptb8[q][:, :].rearrange("p (k n) -> p k n", k=2),
                            wv8[:, q * 2 * D:(q + 1) * 2 * D].rearrange(
                                "p (k n) -> p k n", k=2),
                            start=(q == 0), stop=False, perf_mode=DR,
                        )
                    ic = rb * 4 + j
                    nc.tensor.matmul(
                        o_ps, dg[j], V_nb[:, ic * D:(ic + 1) * D],
                        start=False, stop=True,
                    )
                    o_t = work.tile([128, D], f32, tag="o_t", bufs=4)
                    nc.vector.scalar_tensor_tensor(
                        out=o_t, in0=o_ps, scalar=linv[:, j:j + 1],
                        in1=bv_bc, op0=ALU.mult,
                        op1=ALU.add,
                    )
                    nc.sync.dma_start(
                        out=out[r0 + j * 128: r0 + (j + 1) * 128, :], in_=o_t)

    nc.compile()
    return nc


def _get_nc():
    global _NC
    if _NC is None:
        _NC = build_kernel()
    return _NC


def kernel(embedding, W_qk, b_qk, W_v, b_v):
    global LAST_RESULT
    E = np.ascontiguousarray(np.asarray(embedding, dtype=np.float32))  # [N, F]
    E8 = E.astype(ml_dtypes.float8_e4m3fn)
    chunks = E8.reshape(CC, 128, F)            # (G, p, f) global column chunks

    def prep_w(M):
        M = np.ascontiguousarray(np.asarray(M, dtype=np.float32)).astype(ml_dtypes.bfloat16)
        return np.ascontiguousarray(
            M.reshape(4, 128, M.shape[1]).transpose(1, 0, 2).reshape(128, 4 * M.shape[1]))

    def prep_g(Gm):
        # [p, (g, k, fp, m)] = Gm[fp*128+m, (2g+k)*128+p]
        return np.ascontiguousarray(
            Gm.reshape(FC, 128, 2, 2, 128)      # (fp, m, g, k, p)
            .transpose(4, 2, 3, 0, 1)
            .reshape(128, 2 * 2 * F))

    wqk_f = np.ascontiguousarray(np.asarray(W_qk, dtype=np.float32))
    # weight-only folds: G = 64 * W_qk^T W_qk, h = 64 * W_qk^T b_qk;
    # G ships as two fp8 levels (see module docstring)
    Gm = GSC * (wqk_f.T @ wqk_f)
    G1 = prep_g(Gm).astype(ml_dtypes.float8_e4m3fn)
    G2 = (prep_g(Gm) - G1.astype(np.float32)).astype(ml_dtypes.float8_e4m3fn)
    hv = GSC * (wqk_f.T @ np.asarray(b_qk, dtype=np.float32))
    wvT = np.ascontiguousarray(np.asarray(W_v, dtype=np.float32).T)   # [F, D]
    wv = prep_w(wvT)
    # wv8: [p, (g, k, d)] = 16 * W_v^T[(2g+k)*128+p, d]
    wv8 = np.ascontiguousarray(
        (PSC * wvT).reshape(2, 2, 128, D)       # (g, k, p, d)
        .transpose(2, 0, 1, 3)
        .reshape(128, 2 * 2 * D)).astype(ml_dtypes.float8_e4m3fn)
    bvb = np.ascontiguousarray(
        np.broadcast_to(np.asarray(b_v, dtype=np.float32)[None, :], (128, D)).copy())
    smalls = np.ascontiguousarray(np.concatenate(
        [hv.reshape(FC, 128).T, BIG * np.eye(128, dtype=np.float32)],
        axis=1).astype(np.float32))

    Eb = E.astype(ml_dtypes.bfloat16)
    in_maps = []
    for c in range(CORES):
        order = (np.arange(CC) + 8 * c + 36) % CC
        rot = chunks[order]                     # (q, p, f)
        # et8: [p, (s, fc, q%4, tt)] = E[G(q)*128 + tt, fc*128 + p]
        et8 = np.ascontiguousarray(
            rot.reshape(NSLAB, 4, 128, FC, 128)  # (s, qm, tt, fc, p)
            .transpose(4, 0, 3, 1, 2)
            .reshape(128, NSLAB * FC * SW))
        # en8 (SwInterleave PV stationary): [p, (u, fb, c, i)] =
        #   E[G(2u+i)*128 + p, fb*128 + (127-c)]
        en8 = np.ascontiguousarray(
            rot.reshape(32, 2, 128, FC, 128)[..., ::-1]  # (u, i, p, fb, c)
            .transpose(2, 0, 3, 4, 1)                    # (p, u, fb, c, i)
            .reshape(128, CC * F))
        # etl: [128, (nb, fc, r)] = E[c*NL + nb*512 + r, fc*128+p] in bf16
        etl = np.ascontiguousarray(
            Eb[c * NL:(c + 1) * NL]
            .reshape(RB, SW, FC, 128)
            .transpose(3, 0, 2, 1)
            .reshape(128, RB * FC * SW))
        in_maps.append({
            "et8": et8, "en8": en8, "etl": etl,
            "g8a": G1, "g8b": G2, "wv": wv, "wv8": wv8,
            "smalls": smalls, "bvb": bvb,
        })

    nc = _get_nc()
    res = run_bass_kernel_spmd(nc, in_maps, core_ids=list(range(CORES)))
    LAST_RESULT = res
    return np.concatenate(
        [np.asarray(res.results[i]["out"]) for i in range(CORES)], axis=0
    )


# revision 19
# speedup vs baseline: 1.0337x; 1.0198x over previous
"""Trainium2 8-core attention kernel v15 (N=8192, D=512, Q==K shared projection).

Projection-free formulation; score GEMM, PV GEMM, the output projection AND
the etlg prep all run fp8e4 DoubleRow.

Host folds the weight-only products G = 64*(W_qk^T W_qk) and
h = 64*(W_qk^T b_qk).  G ships as TWO fp8 levels (G1 = f8(64G),
G2 = f8(64G - G1)): single-level fp8 G has a systematic diagonal-logit
shift (e4m3 step at the Gram-diagonal magnitude), two levels recover it;
both levels accumulate into the same PSUM group (fp8 scale-invariance).
The Exp activation scale absorbs the 1/64.

Scores:  st^T[j, i] = 64 * e_j . (G e_i + h)   (row-constant softmax terms
are dropped -- softmax-invariant).  etlg8 = (G1+G2) E8^T_loc + h 1^T comes
straight from the fp8 et8 slabs (slabs 7,8 = the core's own rows, DMA'd
first), so the score stream is gated by only ~640KB of DMA: startup is
DMA-bound (~50-100 GB/s/queue early), hence G1's matmuls run as soon as it
lands (~10us) with their 4 PSUM groups HELD OPEN until G2 arrives (~13us);
scores start ~15us vs ~22us for the bf16-G-via-etl path.

Value side:  attn @ V = (P @ E) W_v^T + b_v.  The softmax DIAGONAL
(logit ~ 10.2, 70% of the softmax mass) is split out exactly: its logit is
extracted from PSUM (masked accum), then suppressed (-2^17) before the Exp
writes P directly in fp8e4.  (P@E)^T runs fp8 DoubleRow over column-chunk
pairs; the projection runs fp8 DoubleRow too (ptb8 = (P@E)^T/16 evac'd
pairwise, wv8 = 16*W_v^T -- scales cancel); the diagonal contribution
p_ii * (E_loc W_v^T)_i is added back from a bf16 local V projection (kept
bf16: the diag mass makes V_loc precision critical) before normalization.
etl (bf16 local E^T) and wv are only needed for V_loc, which is spread one
128-row chunk at a time through u=14..21 of the rb0 score stream; etlg for
row-block 1 is spread through u=2..5.  The u-loop is SOFTWARE-PIPELINED:
score(u+1) issues before PV(u) so the Exp(u) ACT latency hides under real
matmuls (the PE queue is in-order).

The diagonal lands at compile-time-fixed loop positions on every core by
ROTATING each core's column-chunk order: core c processes global chunk
(q + 8c + 36) % 64 at loop position q, so diag chunks are always
q = 28 + rb*4 + jj.  The et8/en8 host buffers are built in that per-core
order (PV/l-sums are order-invariant).

Queues: Q1 (SWDGE/gpsimd, starts ~3us before HWDGE) carries the merged
small-tensor block (h|BIG*I as one [128,132] transfer -- [128, small] DMAs
cost ~2.7us in per-packet overhead, and stride-0 broadcast sources are far
worse, so bv is pre-broadcast on the host), G1, G2 and the early et8/en8
slabs in consumption order; Q0 (HWDGE/sync) carries wv, etl, the late slabs,
bv and all output tiles.  23 junk matmuls bridge the HAM clock-gate warmup
across the DMA-bound startup window.
"""

import ml_dtypes
import numpy as np

import concourse.bass as bass
import concourse.mybir as mybir
import concourse.tile as tile
from concourse import bacc
from concourse.bass_utils import run_bass_kernel_spmd

N = 8192          # sequence length
F = 512           # input features
D = 512           # output features (head dim)
CORES = 8
NL = N // CORES   # local rows per core (1024)
SCALE = 1.0 / float(np.sqrt(D))
GSC = 64.0        # host-side scaling on G/h
SC2 = SCALE / GSC # Exp activation scale (absorbs the G scaling)
PSC = 16.0        # ptb8 = pvt/16, wv8 = 16*W_v^T (scales cancel)
BIG = 131072.0    # 2**17, diagonal suppression constant

FC = F // 128     # 4 f-chunks
DC = D // 128     # 4 d-chunks
RB = NL // 512    # 2 row-blocks of 512
CC = N // 128     # 64 column chunks
SW = 512          # slab width over N for streamed embedding tensors
NSLAB = N // SW   # 16

f32 = mybir.dt.float32
bf16 = mybir.dt.bfloat16
f8 = mybir.dt.float8e4
DR = mybir.MatmulPerfMode.DoubleRow
DRSW = mybir.MatmulPerfMode.DoubleRowSwInterleave
ACT = mybir.ActivationFunctionType
ALU = mybir.AluOpType

_NC = None
LAST_RESULT = None


def build_kernel():
    nc = bacc.Bacc(target_bir_lowering=False)

    # all in exact SBUF layout, host-prepared (et8/en8 per-core chunk-rotated)
    et8d = nc.declare_dram_parameter("et8", [128, NSLAB * FC * SW], f8, isOutput=False)
    en8d = nc.declare_dram_parameter("en8", [128, CC * F], f8, isOutput=False)
    etld = nc.declare_dram_parameter("etl", [128, RB * FC * SW], bf16, isOutput=False)
    g8ad = nc.declare_dram_parameter("g8a", [128, 2 * 2 * F], f8, isOutput=False)
    g8bd = nc.declare_dram_parameter("g8b", [128, 2 * 2 * F], f8, isOutput=False)
    wvd = nc.declare_dram_parameter("wv", [128, FC * D], bf16, isOutput=False)
    wv8d = nc.declare_dram_parameter("wv8", [128, 2 * 2 * D], f8, isOutput=False)
    smd = nc.declare_dram_parameter("smalls", [128, FC + 128], f32, isOutput=False)
    bvbd = nc.declare_dram_parameter("bvb", [128, D], f32, isOutput=False)
    out = nc.declare_dram_parameter("out", [NL, D], f32, isOutput=True)

    with tile.TileContext(nc) as tc:
        with (
            tc.tile_pool(name="persist", bufs=1) as persist,
            tc.tile_pool(name="work", bufs=2) as work,
            tc.tile_pool(name="ps", bufs=3, space="PSUM") as ps,
        ):
            # ---- HAM warmup: junk matmuls keep PE busy while DMAs land ----
            junk = persist.tile([128, 512], bf16)
            nc.vector.memset(junk, 0.25)
            junk_ps = ps.tile([128, 512], f32, tag="mm_ps")
            for _ in range(12):
                nc.tensor.matmul(junk_ps, junk[:, :128], junk,
                                 start=True, stop=True, skip_group_check=True)

            g8a = persist.tile([128, 2 * 2 * F], f8)   # f8(64G), (g,k,fp,m)
            g8b = persist.tile([128, 2 * 2 * F], f8)   # f8(64G - G1)
            wv = persist.tile([128, FC * D], bf16)     # W_v^T,  f-chunk fc at cols fc*D
            wv8 = persist.tile([128, 2 * 2 * D], f8)   # 16*W_v^T, (g,k,d) DR pairs
            # E^T local, nb-major: (nb, fc) block at cols nb*FC*512 + fc*512
            etl = persist.tile([128, RB * FC * SW], bf16)
            et8 = persist.tile([128, NSLAB * FC * SW], f8)
            en8 = persist.tile([128, CC * F], f8)
            smalls = persist.tile([128, FC + 128], f32)  # h | BIG*identity
            h_d = smalls[:, 0:FC]
            idm = smalls[:, FC:FC + 128]
            bv_bc = persist.tile([128, D], f32)

            def et8_slab(sl):
                nc.gpsimd.dma_start(
                    out=et8[:, sl * FC * SW:(sl + 1) * FC * SW],
                    in_=et8d[:, sl * FC * SW:(sl + 1) * FC * SW])

            def en8_slab(sl):
                nc.gpsimd.dma_start(
                    out=en8[:, sl * FC * SW:(sl + 1) * FC * SW],
                    in_=en8d[:, sl * FC * SW:(sl + 1) * FC * SW])

            # Q1 (SWDGE/gpsimd — starts ~3us earlier than HWDGE): the full
            # score-gating chain: smalls, G1, et8 s7 (etlg rhs nb0), G2, s8,
            # s0, s1, en8 s0, then slabs in consumption order
            nc.gpsimd.dma_start(out=g8a[:, :], in_=g8ad[:, :])
            et8_slab(7)
            nc.gpsimd.dma_start(out=g8b[:, :], in_=g8bd[:, :])
            et8_slab(8)
            en8_slab(0)
            et8_slab(2)
            en8_slab(1)
            et8_slab(3)
            en8_slab(2)
            et8_slab(4)
            en8_slab(3)
            en8_slab(4)
            en8_slab(5)
            nc.gpsimd.dma_start(out=wv8[:, :], in_=wv8d[:, :])

            # Q0 (HWDGE/sync — slow start): V_loc inputs, late slabs, bv
            for sl in (0, 1):
                nc.sync.dma_start(
                    out=et8[:, sl * FC * SW:(sl + 1) * FC * SW],
                    in_=et8d[:, sl * FC * SW:(sl + 1) * FC * SW])
            nc.sync.dma_start(out=smalls[:, :], in_=smd[:, :])
            nc.sync.dma_start(out=wv[:, :], in_=wvd[:, :])
            nc.sync.dma_start(out=etl[:, :2048], in_=etld[:, :2048])
            nc.sync.dma_start(out=etl[:, 2048:], in_=etld[:, 2048:])
            for sl in (5, 6, 9, 10):
                nc.sync.dma_start(
                    out=et8[:, sl * FC * SW:(sl + 1) * FC * SW],
                    in_=et8d[:, sl * FC * SW:(sl + 1) * FC * SW])
            for sl in (6, 7):
                nc.sync.dma_start(
                    out=en8[:, sl * FC * SW:(sl + 1) * FC * SW],
                    in_=en8d[:, sl * FC * SW:(sl + 1) * FC * SW])
            nc.sync.dma_start(out=bv_bc, in_=bvbd[:, :])
            for sl in (11, 12, 13, 14, 15):
                nc.sync.dma_start(
                    out=et8[:, sl * FC * SW:(sl + 1) * FC * SW],
                    in_=et8d[:, sl * FC * SW:(sl + 1) * FC * SW])
            for sl in range(8, NSLAB):
                nc.sync.dma_start(
                    out=en8[:, sl * FC * SW:(sl + 1) * FC * SW],
                    in_=en8d[:, sl * FC * SW:(sl + 1) * FC * SW])

            ones_b = persist.tile([128, 1], bf16)
            nc.vector.memset(ones_b, 1.0)

            # ---- prep: etlg8 = (G1+G2) E8^T_loc + h 1^T (fp8 out) ----
            etlg8 = persist.tile([128, FC * NL], f8)    # f-chunk fc at cols fc*NL
            V_nb = persist.tile([128, 8 * D], bf16)     # local V (no bias), ic at ic*D

            def g_lhsT(gt, g, fp):
                return gt[:, g * 2 * F:(g + 1) * 2 * F].rearrange(
                    "p (k n) -> p k n", k=2)[:, :, fp * 128:(fp + 1) * 128]

            def e_rhs(nb, g):
                base = (7 + nb) * FC * SW
                return et8[:, base + 2 * g * SW:
                           base + (2 * g + 2) * SW].rearrange("p (k n) -> p k n", k=2)

            def etlg_evac(ep, nb, fp):
                nc.vector.tensor_scalar_add(
                    out=etlg8[:, fp * NL + nb * 512: fp * NL + nb * 512 + 512],
                    in0=ep, scalar1=h_d[:, fp:fp + 1])

            # nb0: G1 matmuls start as soon as g8a lands; groups stay open
            # until G2 closes them (separate tag: 4 banks held across the wait)
            etlg_ps = [ps.tile([128, 512], f32, tag="pvt_ps", bufs=4,
                               name=f"etlg_ps{fp}") for fp in range(FC)]
            for fp in range(FC):
                for g in range(2):
                    nc.tensor.matmul(etlg_ps[fp], g_lhsT(g8a, g, fp), e_rhs(0, g),
                                     start=(g == 0), stop=False, perf_mode=DR)
            for fp in range(FC):
                for g in range(2):
                    nc.tensor.matmul(etlg_ps[fp], g_lhsT(g8b, g, fp), e_rhs(0, g),
                                     start=False, stop=(g == 1), perf_mode=DR)
                etlg_evac(etlg_ps[fp], 0, fp)

            def emit_etlg_nb1(fp):
                g_ps = ps.tile([128, 512], f32, tag="mm_ps")
                for i, (gt, g) in enumerate(
                        [(g8a, 0), (g8a, 1), (g8b, 0), (g8b, 1)]):
                    nc.tensor.matmul(g_ps, g_lhsT(gt, g, fp), e_rhs(1, g),
                                     start=(i == 0), stop=(i == 3), perf_mode=DR)
                etlg_evac(g_ps, 1, fp)

            def emit_Vnb_ic(ic):
                v_ps = ps.tile([128, 512], f32, tag="mm_ps")
                for fc in range(FC):
                    nc.tensor.matmul(
                        v_ps,
                        etl[:, (ic // 4) * FC * SW + fc * SW + (ic % 4) * 128:
                            (ic // 4) * FC * SW + fc * SW + (ic % 4) * 128 + 128],
                        wv[:, fc * D:(fc + 1) * D],
                        start=(fc == 0), stop=(fc == FC - 1),
                    )
                if ic % 2 == 0:
                    nc.vector.tensor_copy(out=V_nb[:, ic * D:(ic + 1) * D], in_=v_ps)
                else:
                    nc.scalar.activation(out=V_nb[:, ic * D:(ic + 1) * D],
                                         in_=v_ps, func=ACT.Copy)

            # ---- attention: 2 row-blocks of 512 local rows ----
            for rb in range(RB):
                r0 = rb * 512
                pvt_ps = [
                    ps.tile([128, 512], f32, tag="pvt_ps", bufs=4, name=f"pvt{rb}_{fb}")
                    for fb in range(FC)
                ]
                lacc = [work.tile([128, 512], f32, tag="lacc", bufs=4,
                                  name=f"lacc{rb}_{h}") for h in range(2)]
                ppl = work.tile([128, 4], f32, tag="ppl", bufs=2)     # diag logits
                pp4 = work.tile([128, 4], f32, tag="pp4", bufs=2)     # exp(diag)
                dg = [work.tile([128, 128], bf16, tag="dg", bufs=8,
                                name=f"dg_{rb}_{j}") for j in range(4)]
                def emit_score(u):
                    p8 = work.tile([128, 2 * 512], f8, tag="p8", bufs=4)
                    for h in range(2):
                        cc = 2 * u + h
                        sl, t = divmod(cc, FC)
                        st_ps = ps.tile([128, 512], f32, tag="mm_ps")
                        for g in range(2):
                            lhsT = et8[:, sl * FC * SW + 2 * g * SW:
                                       sl * FC * SW + (2 * g + 2) * SW].rearrange(
                                "p (k n) -> p k n", k=2)[:, :, t * 128:(t + 1) * 128]
                            rhs = etlg8[:, 2 * g * NL:(2 * g + 2) * NL].rearrange(
                                "p (k n) -> p k n", k=2)[:, :, r0:r0 + 512]
                            nc.tensor.matmul(
                                st_ps, lhsT, rhs,
                                start=(g == 0), stop=(g == 1), perf_mode=DR,
                            )
                        jj = cc - (28 + rb * 4)
                        if 0 <= jj < 4:
                            # extract diag logit (masked accum), then suppress
                            sli = st_ps[:, jj * 128:(jj + 1) * 128]
                            trash = work.tile([128, 128], f32, tag="trash", bufs=2)
                            nc.vector.scalar_tensor_tensor(
                                out=trash, in0=sli, scalar=1.0 / BIG, in1=idm,
                                op0=ALU.mult, op1=ALU.mult,
                                accum_out=ppl[:, jj:jj + 1],
                            )
                            nc.vector.scalar_tensor_tensor(
                                out=sli, in0=sli, scalar=1.0, in1=idm,
                                op0=ALU.mult, op1=ALU.subtract,
                            )
                        nc.scalar.activation(
                            out=p8[:, h * 512:(h + 1) * 512], in_=st_ps,
                            func=ACT.Exp, scale=SC2,
                        )
                        eng = nc.gpsimd if h == 0 else nc.vector
                        if u == 0:
                            eng.tensor_copy(out=lacc[h], in_=p8[:, h * 512:(h + 1) * 512])
                        else:
                            eng.tensor_add(lacc[h], lacc[h], p8[:, h * 512:(h + 1) * 512])
                        if cc == 31 + rb * 4:
                            nc.scalar.activation(out=pp4, in_=ppl,
                                                 func=ACT.Exp, scale=SC2)
                    return p8

                def emit_pv(u, p8):
                    p8r = p8[:, :].rearrange("p (k n) -> p k n", k=2)
                    for fb in range(FC):
                        base = (u * FC + fb) * 256
                        lhsT = en8[:, base:base + 256].rearrange(
                            "p (c i) -> p c i", i=2)
                        nc.tensor.matmul(
                            pvt_ps[fb], lhsT, p8r,
                            start=(u == 0), stop=(u == CC // 2 - 1), perf_mode=DRSW,
                        )
                    if u == 17 + rb:
                        s2 = work.tile([128, 4], f32, tag="s2", bufs=2)
                        nc.vector.tensor_scalar_mul(out=s2, in0=pp4, scalar1=1.0 / BIG)
                        for j in range(4):
                            nc.vector.tensor_scalar_mul(
                                out=dg[j], in0=idm, scalar1=s2[:, j:j + 1])

                # software-pipelined: score(u+1) issues before PV(u) so the
                # Exp(u) latency hides under score(u+1)'s matmuls (PE queue
                # is in-order)
                prev_p8 = None
                for u in range(CC // 2 + 1):
                    if rb == 0 and 2 <= u < 6:
                        emit_etlg_nb1(u - 2)
                    if rb == 0 and 14 <= u < 22:
                        emit_Vnb_ic(u - 14)
                    p8 = emit_score(u) if u < CC // 2 else None
                    if prev_p8 is not None:
                        emit_pv(u - 1, prev_p8)
                    prev_p8 = p8

                # rb epilogue.  l-sums first (8 tiny matmuls), then
                # (P@E)^T/16 fp8 pair-evacs + fp8 DR W_v projection for all j
                # (keeps PE busy while linv computes on DVE), then
                # normalize+diag-add-back STTs + DMA.
                laccb = [work.tile([128, 512], bf16, tag="laccb", bufs=4,
                                   name=f"laccb{rb}_{h}") for h in range(2)]
                nc.scalar.activation(out=laccb[0], in_=lacc[0], func=ACT.Copy)
                nc.scalar.activation(out=laccb[1], in_=lacc[1], func=ACT.Copy)
                # all (P@E)^T/16 fp8 pair-evacs FIRST so the projection
                # matmuls are never starved behind the l-sum DVE chain
                ptb8 = [
                    [work.tile([128, 256], f8, tag="ptb8", bufs=8,
                               name=f"ptb8_{rb}_{j}_{q}") for q in range(2)]
                    for j in range(4)
                ]
                for j in range(4):
                    for q in range(2):
                        for k in range(2):
                            src_ = pvt_ps[2 * q + k][:, j * 128:(j + 1) * 128]
                            dsl = ptb8[j][q][:, k * 128:(k + 1) * 128]
                            if k == 0:
                                nc.vector.tensor_scalar_mul(
                                    out=dsl, in0=src_, scalar1=1.0 / PSC)
                            else:
                                nc.scalar.activation(
                                    out=dsl, in_=src_, func=ACT.Copy,
                                    scale=1.0 / PSC)
                l_ps = ps.tile([128, 8], f32, tag="l_ps", bufs=1)
                for half in range(2):
                    for j in range(4):
                        nc.tensor.matmul(
                            l_ps[:, half * 4 + j:half * 4 + j + 1],
                            laccb[half][:, j * 128:(j + 1) * 128],
                            ones_b,
                            start=True, stop=True, skip_group_check=True,
                        )
                lsum = work.tile([128, 4], f32, tag="lsum")
                nc.vector.tensor_add(lsum, l_ps[:, 0:4], pp4)
                nc.vector.tensor_add(lsum, lsum, l_ps[:, 4:8])
                linv = work.tile([128, 4], f32, tag="linv")
                nc.vector.reciprocal(out=linv, in_=lsum)
                for j in range(4):
                    o_ps = ps.tile([128, D], f32, tag="mm_ps")
                    for q in range(2):
                        nc.tensor.matmul(
                            o_ps,
                            ptb8[j][q][:, :].rearrange("p (k n) -> p k n", k=2),
                            wv8[:, q * 2 * D:(q + 1) * 2 * D].rearrange(
                                "p (k n) -> p k n", k=2),
                            start=(q == 0), stop=False, perf_mode=DR,
                        )
                    ic = rb * 4 + j
                    nc.tensor.matmul(
                        o_ps, dg[j], V_nb[:, ic * D:(ic + 1) * D],
                        start=False, stop=True,
                    )
                    o_t = work.tile([128, D], f32, tag="o_t", bufs=4)
                    nc.vector.scalar_tensor_tensor(
                        out=o_t, in0=o_ps, scalar=linv[:, j:j + 1],
                        in1=bv_bc, op0=ALU.mult,
                        op1=ALU.add,
                    )
                    nc.sync.dma_start(
                        out=out[r0 + j * 128: r0 + (j + 1) * 128, :], in_=o_t)

    nc.compile()
    return nc


def _get_nc():
    global _NC
    if _NC is None:
        _NC = build_kernel()
    return _NC


def kernel(embedding, W_qk, b_qk, W_v, b_v):
    global LAST_RESULT
    E = np.ascontiguousarray(np.asarray(embedding, dtype=np.float32))  # [N, F]
    E8 = E.astype(ml_dtypes.float8_e4m3fn)
    chunks = E8.reshape(CC, 128, F)            # (G, p, f) global column chunks

    def prep_w(M):
        M = np.ascontiguousarray(np.asarray(M, dtype=np.float32)).astype(ml_dtypes.bfloat16)
        return np.ascontiguousarray(
            M.reshape(4, 128, M.shape[1]).transpose(1, 0, 2).reshape(128, 4 * M.shape[1]))

    def prep_g(Gm):
        # [p, (g, k, fp, m)] = Gm[fp*128+m, (2g+k)*128+p]
        return np.ascontiguousarray(
            Gm.reshape(FC, 128, 2, 2, 128)      # (fp, m, g, k, p)
            .transpose(4, 2, 3, 0, 1)
            .reshape(128, 2 * 2 * F))

    wqk_f = np.ascontiguousarray(np.asarray(W_qk, dtype=np.float32))
    # weight-only folds: G = 64 * W_qk^T W_qk, h = 64 * W_qk^T b_qk;
    # G ships as two fp8 levels (see module docstring)
    Gm = GSC * (wqk_f.T @ wqk_f)
    G1 = prep_g(Gm).astype(ml_dtypes.float8_e4m3fn)
    G2 = (prep_g(Gm) - G1.astype(np.float32)).astype(ml_dtypes.float8_e4m3fn)
    hv = GSC * (wqk_f.T @ np.asarray(b_qk, dtype=np.float32))
    wvT = np.ascontiguousarray(np.asarray(W_v, dtype=np.float32).T)   # [F, D]
    wv = prep_w(wvT)
    # wv8: [p, (g, k, d)] = 16 * W_v^T[(2g+k)*128+p, d]
    wv8 = np.ascontiguousarray(
        (PSC * wvT).reshape(2, 2, 128, D)       # (g, k, p, d)
        .transpose(2, 0, 1, 3)
        .reshape(128, 2 * 2 * D)).astype(ml_dtypes.float8_e4m3fn)
    bvb = np.ascontiguousarray(
        np.broadcast_to(np.asarray(b_v, dtype=np.float32)[None, :], (128, D)).copy())
    smalls = np.ascontiguousarray(np.concatenate(
        [hv.reshape(FC, 128).T, BIG * np.eye(128, dtype=np.float32)],
        axis=1).astype(np.float32))

    Eb = E.astype(ml_dtypes.bfloat16)
    in_maps = []
    for c in range(CORES):
        order = (np.arange(CC) + 8 * c + 36) % CC
        rot = chunks[order]                     # (q, p, f)
        # et8: [p, (s, fc, q%4, tt)] = E[G(q)*128 + tt, fc*128 + p]
        et8 = np.ascontiguousarray(
            rot.reshape(NSLAB, 4, 128, FC, 128)  # (s, qm, tt, fc, p)
            .transpose(4, 0, 3, 1, 2)
            .reshape(128, NSLAB * FC * SW))
        # en8 (SwInterleave PV stationary): [p, (u, fb, c, i)] =
        #   E[G(2u+i)*128 + p, fb*128 + (127-c)]
        en8 = np.ascontiguousarray(
            rot.reshape(32, 2, 128, FC, 128)[..., ::-1]  # (u, i, p, fb, c)
            .transpose(2, 0, 3, 4, 1)                    # (p, u, fb, c, i)
            .reshape(128, CC * F))
        # etl: [128, (nb, fc, r)] = E[c*NL + nb*512 + r, fc*128+p] in bf16
        etl = np.ascontiguousarray(
            Eb[c * NL:(c + 1) * NL]
            .reshape(RB, SW, FC, 128)
            .transpose(3, 0, 2, 1)
            .reshape(128, RB * FC * SW))
        in_maps.append({
            "et8": et8, "en8": en8, "etl": etl,
            "g8a": G1, "g8b": G2, "wv": wv, "wv8": wv8,
            "smalls": smalls, "bvb": bvb,
        })

    nc = _get_nc()
    res = run_bass_kernel_spmd(nc, in_maps, core_ids=list(range(CORES)))
    LAST_RESULT = res
    return np.concatenate(
        [np.asarray(res.results[i]["out"]) for i in range(CORES)], axis=0
    )
